# revision 20
# baseline (speedup 1.0000x reference)
"""Single-head causal attention forward on 8 TRN2 NeuronCores.

Problem: x [8, 2048, 1024] f32, Wq/Wk/Wv [128, 1024] f32.
  q/k/v = x @ W.T ; S = q k^T / sqrt(128) causal ; out = softmax(S) v.

Sharding: data-parallel, one batch element per core (8 cores).
Inside each core a flash-style blocked attention over 512-token chunks:
  - host pre-transposes x[b] into a chunk-major layout [chunk, p, cc, t] so
    the contraction dim (c) lands on SBUF partitions and every DMA piece is
    one contiguous run per partition.
  - qT/kT/vT [h=128, t] via W-stationary matmuls (N=512, weight loads
    hidden); V natural [t, h] via PE transposes of vT into a slotted PSUM
    bank.
  - S^T[j, q] tiles computed full-width (512) in PAIRS into [128,1024] PSUM
    tiles (2 banks); ONE exp ACTIVATE per pair halves ScalarE call overhead.
    A ones-column appended to V makes the PV matmul also produce the softmax
    denominators, so no partition-direction reduction is ever needed.
  - causal masking of the 16 diagonal 128x128 blocks is done by ACCUMULATING
    -1024 into the strictly-masked entries via one extra matmul
    (stationary=eye, rhs=-1024*strict_lower) before the exp: exp maps them
    to ~0, so no DVE mask-multiply and no separate masked tile.
  - PV accumulators live in 3 column-slots of a single PSUM bank (chains
    are strictly sequential on the PE so slot sharing is safe).
  - chunk qc's PV chains are emitted inside chunk qc+1 so PE never waits
    on exp latency except at the tail.
"""

import os
import sys

for _p in ("/opt/trn_rl_repo",):
    if _p not in sys.path and os.path.isdir(_p):
        sys.path.append(_p)

import numpy as np

B, T, D, H = 8, 2048, 1024, 128
CH = 512          # token chunk (free dim of S^T tiles)
NCH = T // CH     # 4 chunks
CC = D // 128     # 8 contraction sub-tiles
NT = T // 128     # 16 token tiles
SCALE = 1.0 / np.sqrt(np.float32(H))
MASKV = -1024.0   # additive pre-scale mask; SCALE*1024 ~ 90 -> exp ~ 1e-39

PROJ_DT = os.environ.get("KERNEL_PROJ_DT", "bfloat16")
ATT_DT = os.environ.get("KERNEL_ATT_DT", "bfloat16")
NWU = int(os.environ.get("KERNEL_NWU", "7"))
# q/k projections in fp8(e4m3) DoubleRow: 2 contraction elems per PE cell,
# halves the q/k projection matmul count. W pre-scaled by QKS into fp8 range;
# S comes out scaled by QKS^2, compensated in the exp scale + mask value.
QK_FP8 = os.environ.get("KERNEL_QK_FP8", "0") == "1"
QKS = 32.0
CCG = 4           # DoubleRow contraction groups of 256
EXP_SCALE = SCALE / (QKS * QKS) if QK_FP8 else SCALE
MSK_HOST = MASKV * ((QKS * QKS) if QK_FP8 else 1.0)

_CACHE = {}


def _build():
    import concourse.bacc as bacc
    import concourse.mybir as mybir
    import concourse.tile as tile

    dt = mybir.dt
    p_dt = getattr(dt, PROJ_DT)
    a_dt = getattr(dt, ATT_DT)

    nc = bacc.Bacc(None)
    xh = nc.declare_dram_parameter("xh", [NCH, 128, CC, CH], p_dt, isOutput=False)
    if QK_FP8:
        f8 = dt.float8e4
        xq8 = nc.declare_dram_parameter("xq8", [NCH, 128, CCG, 2, CH], f8, isOutput=False)
        wq8 = nc.declare_dram_parameter("wq8", [128, CCG, 2, H], f8, isOutput=False)
        wk8 = nc.declare_dram_parameter("wk8", [128, CCG, 2, H], f8, isOutput=False)
    else:
        wqT = nc.declare_dram_parameter("wqT", [128, CC, H], p_dt, isOutput=False)
        wkT = nc.declare_dram_parameter("wkT", [128, CC, H], p_dt, isOutput=False)
    wvT = nc.declare_dram_parameter("wvT", [128, CC, H], p_dt, isOutput=False)
    msk = nc.declare_dram_parameter("msk", [128, 128], a_dt, isOutput=False)
    eye = nc.declare_dram_parameter("eye", [128, 128], a_dt, isOutput=False)
    out = nc.declare_dram_parameter("out", [T, H], dt.float32, isOutput=True)

    with tile.TileContext(nc) as tc:
        with (
            tc.tile_pool(name="singles", bufs=1) as singles,
            tc.tile_pool(name="xp", bufs=3) as xp,
            tc.tile_pool(name="qtp", bufs=2) as qtp,
            tc.tile_pool(name="ktp", bufs=4) as ktp,
            tc.tile_pool(name="vtp", bufs=2) as vtp,
            tc.tile_pool(name="ptp", bufs=18) as ptp,
            tc.tile_pool(name="outp", bufs=4) as outp,
            tc.tile_pool(name="recp", bufs=4) as recp,
            tc.tile_pool(name="psq", bufs=2, space="PSUM") as psq,
            tc.tile_pool(name="pss", bufs=2, space="PSUM") as pss,
            tc.tile_pool(name="psv", bufs=1, space="PSUM") as psv,
            tc.tile_pool(name="pst", bufs=1, space="PSUM") as pst,
        ):
            # PE warmup on zeroed tiles (HAM clock release). memsets on
            # GpSimd so the chain starts right after the framework barrier,
            # while the first x/weight DMAs are still in flight.
            wu_a = singles.tile([128, 128], a_dt)
            wu_b = singles.tile([128, CH], a_dt)
            wu_a_ap, wu_b_ap = wu_a[:], wu_b[:]
            if ATT_DT == "float32r":
                wu_a_ap = wu_a_ap.bitcast(dt.float32)
                wu_b_ap = wu_b_ap.bitcast(dt.float32)
            nc.gpsimd.memset(wu_a_ap, 0.0)
            nc.gpsimd.memset(wu_b_ap, 0.0)
            wu_ps = psq.tile([128, CH], dt.float32, tag="pq")
            for i in range(NWU):
                nc.tensor.matmul(
                    wu_ps[:], wu_a[:], wu_b[:],
                    start=(i == 0), stop=(i == NWU - 1),
                )

            # --- weights / constants: one queue per tensor so the triggers
            # don't serialize and wk/wv aren't stuck behind the x stream ---
            if QK_FP8:
                wq_sb = singles.tile([128, CCG, 2, H], f8)
                wk_sb = singles.tile([128, CCG, 2, H], f8)
                for g0 in range(0, CCG, 2):
                    nc.gpsimd.dma_start(
                        out=wq_sb[:, g0 : g0 + 2], in_=wq8[:, g0 : g0 + 2]
                    )
                nc.scalar.dma_start(out=wk_sb[:], in_=wk8[:])
            else:
                wq_sb = singles.tile([128, CC, H], p_dt)
                wk_sb = singles.tile([128, CC, H], p_dt)
                for g0 in range(0, CC, 2):
                    nc.gpsimd.dma_start(
                        out=wq_sb[:, g0 : g0 + 2, :], in_=wqT[:, g0 : g0 + 2, :]
                    )
                nc.scalar.dma_start(out=wk_sb[:], in_=wkT[:])
            wv_sb = singles.tile([128, CC, H], p_dt)
            eye_sb = singles.tile([128, 128], a_dt)
            msk_sb = singles.tile([128, 128], a_dt)
            nc.scalar.dma_start(out=wv_sb[:], in_=wvT[:])
            nc.gpsimd.dma_start(out=eye_sb[:], in_=eye[:])
            nc.gpsimd.dma_start(out=msk_sb[:], in_=msk[:])

            # V' = [V | 1]; ones columns written once
            v_sb = singles.tile([128, NT, H + 4], a_dt)
            ones_ap = v_sb[:, :, H : H + 2]
            if ATT_DT == "float32r":
                ones_ap = ones_ap.bitcast(dt.float32)
            nc.vector.memset(ones_ap, 1.0)

            # persistent slotted PSUM banks
            pv_acc = psv.tile([128, 512], dt.float32)   # 3 slots x 130
            tr_acc = pst.tile([128, 512], a_dt)         # 4 slots x 128

            kt_tiles = []
            pts_all = []   # per chunk: list of pt pair tiles [128, 2*CH]

            def emit_chains(qc):
                """PV chains + normalize + store for q-chunk qc."""
                pts_c = pts_all[qc]
                for ti in range(4):
                    qi = qc * 4 + ti
                    slot = qi % 3
                    ops = pv_acc[:, slot * 130 : slot * 130 + 130]
                    for j2 in range(qi + 1):
                        pt_pair = pts_c[j2 // 2]
                        c0 = (j2 % 2) * CH + ti * 128
                        nc.tensor.matmul(
                            ops[:, 0 : H + 2],
                            pt_pair[:, c0 : c0 + 128],
                            v_sb[:, j2, 0 : H + 2],
                            start=(j2 == 0), stop=(j2 == qi),
                        )
                    rec = recp.tile([128, 1], dt.float32)
                    nc.vector.reciprocal(rec[:], ops[:, H : H + 1])
                    ob = outp.tile([128, H], dt.float32)
                    # normalize on ScalarE (Copy act, per-partition scale AP):
                    # keeps the DVE queue clear so PV slots recycle faster
                    nc.scalar.mul(ob[:], ops[:, 0:H], rec[:])
                    eng = nc.sync if (qi % 2 == 0) else nc.gpsimd
                    eng.dma_start(
                        out=out[qi * 128 : (qi + 1) * 128, :], in_=ob[:]
                    )

            xts = {}
            qts = {}

            def emit_xload(qc):
                """x chunk DMA; chunk 0 in small pieces so the first
                projection matmuls start as soon as possible."""
                if QK_FP8:
                    xt8 = xp.tile([128, CCG, 2, CH], f8, tag="x8")
                    b8 = [0, 1, 2, 3, 4] if qc == 0 else [0, 2, 4]
                    for a, b in zip(b8, b8[1:]):
                        nc.sync.dma_start(out=xt8[:, a:b], in_=xq8[qc, :, a:b])
                    xts[("x8", qc)] = xt8
                xt = xp.tile([128, CC, CH], p_dt)
                bounds = ([0, 2, 4, 6, 8] if QK_FP8 else [0, 1, 2, 3, 4, 6, 8]) \
                    if qc == 0 else [0, 4, 8]
                for a, b in zip(bounds, bounds[1:]):
                    nc.sync.dma_start(out=xt[:, a:b, :], in_=xh[qc, :, a:b, :])
                xts[qc] = xt

            def _proj(w_sb, xt, xt8):
                ps = psq.tile([128, CH], dt.float32, tag="pq", name="ps")
                if QK_FP8:
                    for g in range(CCG):
                        nc.tensor.matmul(
                            ps[:], w_sb[:, g], xt8[:, g],
                            start=(g == 0), stop=(g == CCG - 1),
                            perf_mode=mybir.MatmulPerfMode.DoubleRow,
                        )
                else:
                    for cc in range(CC):
                        nc.tensor.matmul(
                            ps[:], w_sb[:, cc, :], xt[:, cc, :],
                            start=(cc == 0), stop=(cc == CC - 1),
                        )
                return ps

            def emit_qkproj(qc):
                xt, xt8 = xts[qc], xts.get(("x8", qc))
                qps = _proj(wq_sb, xt, xt8)
                qt = qtp.tile([128, CH], a_dt)
                nc.vector.tensor_copy(qt[:], qps[:])
                qts[qc] = qt
                kps = _proj(wk_sb, xt, xt8)
                kt = ktp.tile([128, CH], a_dt)
                nc.vector.tensor_copy(kt[:], kps[:])
                kt_tiles.append(kt)

            def emit_spairs(qc, p0, p1):
                """S^T pairs [p0, p1): two full-width j-tiles per [128,1024]
                PSUM tile, one exp per pair. Diagonal blocks get -1024 added
                to their strictly-masked entries via an extra matmul."""
                qt = qts[qc]
                pts_c = pts_all[qc]
                for p in range(p0, p1):
                    sp = pss.tile([128, 2 * CH], dt.float32)
                    pt = ptp.tile([128, 2 * CH], a_dt)
                    diag_pair = 2 * p >= qc * 4
                    for hh in range(2):
                        jt = 2 * p + hh
                        kt_src = kt_tiles[jt // 4]
                        v0 = (jt - qc * 4) * 128 if diag_pair else 0
                        nc.tensor.matmul(
                            sp[:, hh * CH + v0 : (hh + 1) * CH],
                            kt_src[:, (jt % 4) * 128 : (jt % 4 + 1) * 128],
                            qt[:, v0:CH],
                            start=True, stop=not diag_pair,
                        )
                        if diag_pair:
                            b0 = hh * CH + v0
                            nc.tensor.matmul(
                                sp[:, b0 : b0 + 128],
                                eye_sb[:], msk_sb[:],
                                start=False, stop=True,
                            )
                            nc.scalar.activation(
                                pt[:, b0 : (hh + 1) * CH],
                                sp[:, b0 : (hh + 1) * CH],
                                mybir.ActivationFunctionType.Exp,
                                scale=float(EXP_SCALE),
                            )
                    if not diag_pair:
                        nc.scalar.activation(
                            pt[:], sp[:], mybir.ActivationFunctionType.Exp,
                            scale=float(EXP_SCALE),
                        )
                    pts_c.append(pt)

            def emit_vproj(qc):
                xt = xts[qc]
                vps = psq.tile([128, CH], dt.float32, tag="pq", name="vps")
                for cc in range(CC):
                    nc.tensor.matmul(
                        vps[:], wv_sb[:, cc, :], xt[:, cc, :],
                        start=(cc == 0), stop=(cc == CC - 1),
                    )
                vt = vtp.tile([128, CH], a_dt)
                nc.vector.tensor_copy(vt[:], vps[:])
                for ti in range(4):
                    jt = qc * 4 + ti
                    dst = tr_acc[:, ti * 128 : (ti + 1) * 128]
                    nc.tensor.transpose(
                        dst, vt[:, ti * 128 : (ti + 1) * 128], eye_sb[:]
                    )
                    nc.vector.tensor_copy(v_sb[:, jt, 0:H], dst)

            pts_all.extend([[] for _ in range(NCH)])
            # Explicit schedule: chunk qc's PV chains run one chunk later so
            # the PE never waits on exp latency; chunk 3's off-diagonal S
            # pairs are pulled into chunk 2 so only its two diagonal pairs
            # gate the final chains.
            for qc in range(3):
                emit_xload(qc)
            emit_qkproj(0); emit_spairs(0, 0, 2); emit_vproj(0)
            emit_qkproj(1); emit_spairs(1, 0, 4); emit_chains(0); emit_vproj(1)
            emit_qkproj(2); emit_spairs(2, 0, 6)
            emit_xload(3)
            emit_qkproj(3); emit_spairs(3, 0, 6)
            emit_chains(1); emit_vproj(2)
            emit_chains(2); emit_spairs(3, 6, 8); emit_vproj(3)
            emit_chains(3)

    nc.compile()
    return nc


def _get_nc():
    if "nc" not in _CACHE:
        _CACHE["nc"] = _build()
    return _CACHE["nc"]


def _np_dt(name):
    if name == "bfloat16":
        import ml_dtypes

        return ml_dtypes.bfloat16
    return np.float32


def _in_maps(x, Wq, Wk, Wv):
    pdt = _np_dt(PROJ_DT)
    adt = _np_dt(ATT_DT)

    def _wprep(W):
        # W [H, D] -> [128p, CC, H] with per-partition-contiguous rows
        WT = np.asarray(W, dtype=np.float32).T.reshape(CC, 128, H)
        return np.ascontiguousarray(WT.transpose(1, 0, 2)).astype(pdt)

    wv = _wprep(Wv)
    # msk[j, q] = MSK_HOST where q < j (strictly masked in the diagonal block)
    msk = (MSK_HOST * np.tril(np.ones((128, 128), dtype=np.float32), -1)).astype(adt)
    eye = np.eye(128, dtype=np.float32).astype(adt)
    x = np.asarray(x, dtype=np.float32)

    if QK_FP8:
        import ml_dtypes

        f8 = getattr(ml_dtypes, "float8_e4m3", None) or ml_dtypes.float8_e4m3fn

        def _wprep8(W):
            WT = (np.float32(QKS) * np.asarray(W, np.float32)).T  # [D, H]
            WT = WT.reshape(CCG, 2, 128, H).transpose(2, 0, 1, 3)
            return np.ascontiguousarray(WT).astype(f8)

        wq8, wk8 = _wprep8(Wq), _wprep8(Wk)
    else:
        wq, wk = _wprep(Wq), _wprep(Wk)

    maps = []
    for b in range(B):
        # [qc, p, cc, t]: per (qc, p) a contiguous CC*CH run
        xh = np.ascontiguousarray(
            x[b].T.reshape(CC, 128, NCH, CH).transpose(2, 1, 0, 3)
        ).astype(pdt)
        m = {"xh": xh, "wvT": wv, "msk": msk, "eye": eye}
        if QK_FP8:
            m["xq8"] = np.ascontiguousarray(
                x[b].T.reshape(CCG, 2, 128, NCH, CH).transpose(3, 2, 0, 1, 4)
            ).astype(f8)
            m["wq8"], m["wk8"] = wq8, wk8
        else:
            m["wqT"], m["wkT"] = wq, wk
        maps.append(m)
    return maps


def kernel(x, Wq, Wk, Wv):
    from concourse.bass_utils import run_bass_kernel_spmd

    nc = _get_nc()
    res = run_bass_kernel_spmd(nc, _in_maps(x, Wq, Wk, Wv), core_ids=list(range(B)))
    return np.stack([res.results[b]["out"] for b in range(B)]).astype(np.float32)


# revision 22
# speedup vs baseline: 1.2288x; 1.2288x over previous
"""Single-head causal attention forward on 8 TRN2 NeuronCores.

Problem: x [8, 2048, 1024] f32, Wq/Wk/Wv [128, 1024] f32.
  q/k/v = x @ W.T ; S = q k^T / sqrt(128) causal ; out = softmax(S) v.

Sharding: data-parallel, one batch element per core (8 cores).
Inside each core a flash-style blocked attention over 512-token chunks:
  - host pre-transposes x[b] into a chunk-major layout [chunk, p, cc, t] so
    the contraction dim (c) lands on SBUF partitions and every DMA piece is
    one contiguous run per partition.
  - qT/kT/vT [h=128, t] via W-stationary matmuls (N=512, weight loads
    hidden); V natural [t, h] via PE transposes of vT into a slotted PSUM
    bank.
  - S^T[j, q] tiles computed full-width (512) in PAIRS into [128,1024] PSUM
    tiles (2 banks); ONE exp ACTIVATE per pair halves ScalarE call overhead.
    A ones-column appended to V makes the PV matmul also produce the softmax
    denominators, so no partition-direction reduction is ever needed.
  - causal masking of the 16 diagonal 128x128 blocks is done by ACCUMULATING
    -1024 into the strictly-masked entries via one extra matmul
    (stationary=eye, rhs=-1024*strict_lower) before the exp: exp maps them
    to ~0, so no DVE mask-multiply and no separate masked tile.
  - PV accumulators live in 3 column-slots of a single PSUM bank (chains
    are strictly sequential on the PE so slot sharing is safe).
  - chunk qc's PV chains are emitted inside chunk qc+1 so PE never waits
    on exp latency except at the tail.
"""

import os
import sys

for _p in ("/opt/trn_rl_repo",):
    if _p not in sys.path and os.path.isdir(_p):
        sys.path.append(_p)

import numpy as np

B, T, D, H = 8, 2048, 1024, 128
CH = 512          # token chunk (free dim of S^T tiles)
NCH = T // CH     # 4 chunks
CC = D // 128     # 8 contraction sub-tiles
NT = T // 128     # 16 token tiles
SCALE = 1.0 / np.sqrt(np.float32(H))
MASKV = -1024.0   # additive pre-scale mask; SCALE*1024 ~ 90 -> exp ~ 1e-39

PROJ_DT = os.environ.get("KERNEL_PROJ_DT", "bfloat16")
ATT_DT = os.environ.get("KERNEL_ATT_DT", "bfloat16")
NWU = int(os.environ.get("KERNEL_NWU", "7"))
# q/k projections in fp8(e4m3) DoubleRow: 2 contraction elems per PE cell,
# halves the q/k projection matmul count. W pre-scaled by QKS into fp8 range;
# S comes out scaled by QKS^2, compensated in the exp scale + mask value.
QK_FP8 = os.environ.get("KERNEL_QK_FP8", "0") == "1"
QKS = 32.0
CCG = 4           # DoubleRow contraction groups of 256
EXP_SCALE = SCALE / (QKS * QKS) if QK_FP8 else SCALE
MSK_HOST = MASKV * ((QKS * QKS) if QK_FP8 else 1.0)

_CACHE = {}


def _build():
    import concourse.bacc as bacc
    import concourse.mybir as mybir
    import concourse.tile as tile

    dt = mybir.dt
    p_dt = getattr(dt, PROJ_DT)
    a_dt = getattr(dt, ATT_DT)

    nc = bacc.Bacc(None)
    xh = nc.declare_dram_parameter("xh", [NCH, 128, CC, CH], p_dt, isOutput=False)
    if QK_FP8:
        f8 = dt.float8e4
        xq8 = nc.declare_dram_parameter("xq8", [NCH, 128, CCG, 2, CH], f8, isOutput=False)
        wq8 = nc.declare_dram_parameter("wq8", [128, CCG, 2, H], f8, isOutput=False)
        wk8 = nc.declare_dram_parameter("wk8", [128, CCG, 2, H], f8, isOutput=False)
    else:
        wqT = nc.declare_dram_parameter("wqT", [128, CC, H], p_dt, isOutput=False)
        wkT = nc.declare_dram_parameter("wkT", [128, CC, H], p_dt, isOutput=False)
    wvT = nc.declare_dram_parameter("wvT", [128, CC, H], p_dt, isOutput=False)
    msk = nc.declare_dram_parameter("msk", [128, 128], a_dt, isOutput=False)
    eye = nc.declare_dram_parameter("eye", [128, 128], a_dt, isOutput=False)
    out = nc.declare_dram_parameter("out", [T, H], dt.float32, isOutput=True)

    with tile.TileContext(nc) as tc:
        with (
            tc.tile_pool(name="singles", bufs=1) as singles,
            tc.tile_pool(name="xp", bufs=3) as xp,
            tc.tile_pool(name="qtp", bufs=2) as qtp,
            tc.tile_pool(name="ktp", bufs=4) as ktp,
            tc.tile_pool(name="vtp", bufs=2) as vtp,
            tc.tile_pool(name="ptp", bufs=18) as ptp,
            tc.tile_pool(name="outp", bufs=4) as outp,
            tc.tile_pool(name="recp", bufs=4) as recp,
            tc.tile_pool(name="psq", bufs=2, space="PSUM") as psq,
            tc.tile_pool(name="pss", bufs=2, space="PSUM") as pss,
            tc.tile_pool(name="psv", bufs=1, space="PSUM") as psv,
            tc.tile_pool(name="pst", bufs=1, space="PSUM") as pst,
        ):
            # PE warmup on zeroed tiles (HAM clock release). memsets on
            # GpSimd so the chain starts right after the framework barrier,
            # while the first x/weight DMAs are still in flight.
            wu_a = singles.tile([128, 128], a_dt)
            wu_b = singles.tile([128, CH], a_dt)
            wu_a_ap, wu_b_ap = wu_a[:], wu_b[:]
            if ATT_DT == "float32r":
                wu_a_ap = wu_a_ap.bitcast(dt.float32)
                wu_b_ap = wu_b_ap.bitcast(dt.float32)
            nc.gpsimd.memset(wu_a_ap, 0.0)
            nc.gpsimd.memset(wu_b_ap, 0.0)
            wu_ps = psq.tile([128, CH], dt.float32, tag="pq")
            for i in range(NWU):
                nc.tensor.matmul(
                    wu_ps[:], wu_a[:], wu_b[:],
                    start=(i == 0), stop=(i == NWU - 1),
                )

            # --- weights / constants: one queue per tensor so the triggers
            # don't serialize and wk/wv aren't stuck behind the x stream ---
            if QK_FP8:
                wq_sb = singles.tile([128, CCG, 2, H], f8)
                wk_sb = singles.tile([128, CCG, 2, H], f8)
                for g0 in range(0, CCG, 2):
                    nc.gpsimd.dma_start(
                        out=wq_sb[:, g0 : g0 + 2], in_=wq8[:, g0 : g0 + 2]
                    )
                nc.scalar.dma_start(out=wk_sb[:], in_=wk8[:])
            else:
                wq_sb = singles.tile([128, CC, H], p_dt)
                wk_sb = singles.tile([128, CC, H], p_dt)
                for g0 in range(0, CC, 2):
                    nc.gpsimd.dma_start(
                        out=wq_sb[:, g0 : g0 + 2, :], in_=wqT[:, g0 : g0 + 2, :]
                    )
                nc.scalar.dma_start(out=wk_sb[:], in_=wkT[:])
            wv_sb = singles.tile([128, CC, H], p_dt)
            eye_sb = singles.tile([128, 128], a_dt)
            msk_sb = singles.tile([128, 128], a_dt)
            nc.scalar.dma_start(out=wv_sb[:], in_=wvT[:])
            nc.gpsimd.dma_start(out=eye_sb[:], in_=eye[:])
            nc.gpsimd.dma_start(out=msk_sb[:], in_=msk[:])

            # V' = [V | 1]; ones columns written once
            v_sb = singles.tile([128, NT, H + 4], a_dt)
            ones_ap = v_sb[:, :, H : H + 2]
            if ATT_DT == "float32r":
                ones_ap = ones_ap.bitcast(dt.float32)
            nc.vector.memset(ones_ap, 1.0)

            # persistent slotted PSUM banks
            pv_acc = psv.tile([128, 512], dt.float32)   # 3 slots x 130
            tr_acc = pst.tile([128, 512], a_dt)         # 4 slots x 128

            kt_tiles = []
            pts_all = []   # per chunk: list of pt pair tiles [128, 2*CH]

            def emit_chains(qc):
                """PV chains + normalize + store for q-chunk qc. The last
                chunk's chains mostly use the (by then idle) psq banks: a
                chain's start=True clobbers its bank's zero-region, so the
                framework serializes it behind all readers of that bank —
                alternating banks keeps the tail chains off that WAR."""
                pts_c = pts_all[qc]
                for ti in range(4):
                    qi = qc * 4 + ti
                    if qc == NCH - 1 and ti != 2:
                        opst = psq.tile([128, CH], dt.float32, tag="pq",
                                        name="opst")
                        ops = opst[:, 0:130]
                    else:
                        slot = qi % 3
                        ops = pv_acc[:, slot * 130 : slot * 130 + 130]
                    for j2 in range(qi + 1):
                        pt_pair = pts_c[j2 // 2]
                        c0 = (j2 % 2) * CH + ti * 128
                        nc.tensor.matmul(
                            ops[:, 0 : H + 2],
                            pt_pair[:, c0 : c0 + 128],
                            v_sb[:, j2, 0 : H + 2],
                            start=(j2 == 0), stop=(j2 == qi),
                        )
                    rec = recp.tile([128, 1], dt.float32)
                    nc.vector.reciprocal(rec[:], ops[:, H : H + 1])
                    ob = outp.tile([128, H], dt.float32)
                    nc.vector.tensor_scalar_mul(ob[:], ops[:, 0:H], rec[:])
                    eng = nc.sync if (qi % 2 == 0) else nc.gpsimd
                    eng.dma_start(
                        out=out[qi * 128 : (qi + 1) * 128, :], in_=ob[:]
                    )

            xts = {}
            qts = {}

            def emit_xload(qc):
                """x chunk DMA; chunk 0 in small pieces so the first
                projection matmuls start as soon as possible."""
                if QK_FP8:
                    xt8 = xp.tile([128, CCG, 2, CH], f8, tag="x8")
                    b8 = [0, 1, 2, 3, 4] if qc == 0 else [0, 2, 4]
                    for a, b in zip(b8, b8[1:]):
                        nc.sync.dma_start(out=xt8[:, a:b], in_=xq8[qc, :, a:b])
                    xts[("x8", qc)] = xt8
                xt = xp.tile([128, CC, CH], p_dt)
                bounds = ([0, 2, 4, 6, 8] if QK_FP8 else [0, 1, 2, 3, 4, 6, 8]) \
                    if qc == 0 else [0, 4, 8]
                for a, b in zip(bounds, bounds[1:]):
                    nc.sync.dma_start(out=xt[:, a:b, :], in_=xh[qc, :, a:b, :])
                xts[qc] = xt

            def _proj(w_sb, xt, xt8):
                ps = psq.tile([128, CH], dt.float32, tag="pq", name="ps")
                if QK_FP8:
                    for g in range(CCG):
                        nc.tensor.matmul(
                            ps[:], w_sb[:, g], xt8[:, g],
                            start=(g == 0), stop=(g == CCG - 1),
                            perf_mode=mybir.MatmulPerfMode.DoubleRow,
                        )
                else:
                    for cc in range(CC):
                        nc.tensor.matmul(
                            ps[:], w_sb[:, cc, :], xt[:, cc, :],
                            start=(cc == 0), stop=(cc == CC - 1),
                        )
                return ps

            def emit_qkproj(qc):
                xt, xt8 = xts[qc], xts.get(("x8", qc))
                qps = _proj(wq_sb, xt, xt8)
                qt = qtp.tile([128, CH], a_dt)
                nc.vector.tensor_copy(qt[:], qps[:])
                qts[qc] = qt
                kps = _proj(wk_sb, xt, xt8)
                kt = ktp.tile([128, CH], a_dt)
                nc.vector.tensor_copy(kt[:], kps[:])
                kt_tiles.append(kt)

            def emit_spairs(qc, p0, p1):
                """S^T pairs [p0, p1): two full-width j-tiles per [128,1024]
                PSUM tile, one exp per pair. Diagonal blocks get -1024 added
                to their strictly-masked entries via an extra matmul."""
                qt = qts[qc]
                pts_c = pts_all[qc]
                for p in range(p0, p1):
                    sp = pss.tile([128, 2 * CH], dt.float32)
                    pt = ptp.tile([128, 2 * CH], a_dt)
                    diag_pair = 2 * p >= qc * 4
                    for hh in range(2):
                        jt = 2 * p + hh
                        kt_src = kt_tiles[jt // 4]
                        v0 = (jt - qc * 4) * 128 if diag_pair else 0
                        nc.tensor.matmul(
                            sp[:, hh * CH + v0 : (hh + 1) * CH],
                            kt_src[:, (jt % 4) * 128 : (jt % 4 + 1) * 128],
                            qt[:, v0:CH],
                            start=True, stop=not diag_pair,
                        )
                        if diag_pair:
                            b0 = hh * CH + v0
                            nc.tensor.matmul(
                                sp[:, b0 : b0 + 128],
                                eye_sb[:], msk_sb[:],
                                start=False, stop=True,
                            )
                            nc.scalar.activation(
                                pt[:, b0 : (hh + 1) * CH],
                                sp[:, b0 : (hh + 1) * CH],
                                mybir.ActivationFunctionType.Exp,
                                scale=float(EXP_SCALE),
                            )
                    if not diag_pair:
                        nc.scalar.activation(
                            pt[:], sp[:], mybir.ActivationFunctionType.Exp,
                            scale=float(EXP_SCALE),
                        )
                    pts_c.append(pt)

            def emit_vproj(qc):
                xt = xts[qc]
                vps = psq.tile([128, CH], dt.float32, tag="pq", name="vps")
                for cc in range(CC):
                    nc.tensor.matmul(
                        vps[:], wv_sb[:, cc, :], xt[:, cc, :],
                        start=(cc == 0), stop=(cc == CC - 1),
                    )
                vt = vtp.tile([128, CH], a_dt)
                nc.vector.tensor_copy(vt[:], vps[:])
                for ti in range(4):
                    jt = qc * 4 + ti
                    dst = tr_acc[:, ti * 128 : (ti + 1) * 128]
                    nc.tensor.transpose(
                        dst, vt[:, ti * 128 : (ti + 1) * 128], eye_sb[:]
                    )
                    nc.vector.tensor_copy(v_sb[:, jt, 0:H], dst)

            pts_all.extend([[] for _ in range(NCH)])
            # Explicit schedule: chunk qc's PV chains run one chunk later so
            # the PE never waits on exp latency; chunk 3's off-diagonal S
            # pairs are pulled into chunk 2 so only its two diagonal pairs
            # gate the final chains.
            for qc in range(3):
                emit_xload(qc)
            emit_qkproj(0); emit_spairs(0, 0, 2); emit_vproj(0)
            emit_qkproj(1); emit_spairs(1, 0, 4); emit_chains(0); emit_vproj(1)
            emit_qkproj(2); emit_spairs(2, 0, 6)
            emit_xload(3)
            emit_qkproj(3); emit_spairs(3, 0, 6)
            emit_chains(1); emit_vproj(2)
            emit_chains(2); emit_spairs(3, 6, 8); emit_vproj(3)
            emit_chains(3)

    nc.compile()
    return nc


def _get_nc():
    if "nc" not in _CACHE:
        _CACHE["nc"] = _build()
    return _CACHE["nc"]


def _np_dt(name):
    if name == "bfloat16":
        import ml_dtypes

        return ml_dtypes.bfloat16
    return np.float32


def _in_maps(x, Wq, Wk, Wv):
    pdt = _np_dt(PROJ_DT)
    adt = _np_dt(ATT_DT)

    def _wprep(W):
        # W [H, D] -> [128p, CC, H] with per-partition-contiguous rows
        WT = np.asarray(W, dtype=np.float32).T.reshape(CC, 128, H)
        return np.ascontiguousarray(WT.transpose(1, 0, 2)).astype(pdt)

    wv = _wprep(Wv)
    # msk[j, q] = MSK_HOST where q < j (strictly masked in the diagonal block)
    msk = (MSK_HOST * np.tril(np.ones((128, 128), dtype=np.float32), -1)).astype(adt)
    eye = np.eye(128, dtype=np.float32).astype(adt)
    x = np.asarray(x, dtype=np.float32)

    if QK_FP8:
        import ml_dtypes

        f8 = getattr(ml_dtypes, "float8_e4m3", None) or ml_dtypes.float8_e4m3fn

        def _wprep8(W):
            WT = (np.float32(QKS) * np.asarray(W, np.float32)).T  # [D, H]
            WT = WT.reshape(CCG, 2, 128, H).transpose(2, 0, 1, 3)
            return np.ascontiguousarray(WT).astype(f8)

        wq8, wk8 = _wprep8(Wq), _wprep8(Wk)
    else:
        wq, wk = _wprep(Wq), _wprep(Wk)

    maps = []
    for b in range(B):
        # [qc, p, cc, t]: per (qc, p) a contiguous CC*CH run
        xh = np.ascontiguousarray(
            x[b].T.reshape(CC, 128, NCH, CH).transpose(2, 1, 0, 3)
        ).astype(pdt)
        m = {"xh": xh, "wvT": wv, "msk": msk, "eye": eye}
        if QK_FP8:
            m["xq8"] = np.ascontiguousarray(
                x[b].T.reshape(CCG, 2, 128, NCH, CH).transpose(3, 2, 0, 1, 4)
            ).astype(f8)
            m["wq8"], m["wk8"] = wq8, wk8
        else:
            m["wqT"], m["wkT"] = wq, wk
        maps.append(m)
    return maps


def kernel(x, Wq, Wk, Wv):
    from concourse.bass_utils import run_bass_kernel_spmd

    nc = _get_nc()
    res = run_bass_kernel_spmd(nc, _in_maps(x, Wq, Wk, Wv), core_ids=list(range(B)))
    return np.stack([res.results[b]["out"] for b in range(B)]).astype(np.float32)


# revision 26
# speedup vs baseline: 1.3932x; 1.1338x over previous
"""Single-head causal attention forward on 8 TRN2 NeuronCores.

Problem: x [8, 2048, 1024] f32, Wq/Wk/Wv [128, 1024] f32.
  q/k/v = x @ W.T ; S = q k^T / sqrt(128) causal ; out = softmax(S) v.

Sharding: data-parallel, one batch element per core (8 cores).
Inside each core a flash-style blocked attention over 512-token chunks:
  - host pre-transposes x[b] into a chunk-major layout [chunk, p, cc, t] so
    the contraction dim (c) lands on SBUF partitions and every DMA piece is
    one contiguous run per partition.
  - qT/kT/vT [h=128, t] via W-stationary matmuls (N=512, weight loads
    hidden); V natural [t, h] via PE transposes of vT into a slotted PSUM
    bank.
  - S^T[j, q] tiles computed full-width (512) in PAIRS into [128,1024] PSUM
    tiles (2 banks); ONE exp ACTIVATE per pair halves ScalarE call overhead.
    A ones-column appended to V makes the PV matmul also produce the softmax
    denominators, so no partition-direction reduction is ever needed.
  - causal masking of the 16 diagonal 128x128 blocks is done by ACCUMULATING
    -1024 into the strictly-masked entries via one extra matmul
    (stationary=eye, rhs=-1024*strict_lower) before the exp: exp maps them
    to ~0, so no DVE mask-multiply and no separate masked tile.
  - PV accumulators live in 3 column-slots of a single PSUM bank (chains
    are strictly sequential on the PE so slot sharing is safe).
  - chunk qc's PV chains are emitted inside chunk qc+1 so PE never waits
    on exp latency except at the tail.
"""

import os
import sys

for _p in ("/opt/trn_rl_repo",):
    if _p not in sys.path and os.path.isdir(_p):
        sys.path.append(_p)

import numpy as np

B, T, D, H = 8, 2048, 1024, 128
CH = 512          # token chunk (free dim of S^T tiles)
NCH = T // CH     # 4 chunks
CC = D // 128     # 8 contraction sub-tiles
NT = T // 128     # 16 token tiles
SCALE = 1.0 / np.sqrt(np.float32(H))
MASKV = -1024.0   # additive pre-scale mask; SCALE*1024 ~ 90 -> exp ~ 1e-39

PROJ_DT = os.environ.get("KERNEL_PROJ_DT", "bfloat16")
ATT_DT = os.environ.get("KERNEL_ATT_DT", "bfloat16")
NWU = int(os.environ.get("KERNEL_NWU", "7"))
# q/k projections in fp8(e4m3) DoubleRow: 2 contraction elems per PE cell,
# halves the q/k projection matmul count. W pre-scaled by QKS into fp8 range;
# S comes out scaled by QKS^2, compensated in the exp scale + mask value.
QK_FP8 = os.environ.get("KERNEL_QK_FP8", "0") == "1"
QKS = 32.0
CCG = 4           # DoubleRow contraction groups of 256
EXP_SCALE = SCALE / (QKS * QKS) if QK_FP8 else SCALE
MSK_HOST = MASKV * ((QKS * QKS) if QK_FP8 else 1.0)

_CACHE = {}


def _build():
    import concourse.bacc as bacc
    import concourse.mybir as mybir
    import concourse.tile as tile

    dt = mybir.dt
    p_dt = getattr(dt, PROJ_DT)
    a_dt = getattr(dt, ATT_DT)

    nc = bacc.Bacc(None)
    xh = nc.declare_dram_parameter("xh", [NCH, 128, CC, CH], p_dt, isOutput=False)
    if QK_FP8:
        f8 = dt.float8e4
        xq8 = nc.declare_dram_parameter("xq8", [NCH, 128, CCG, 2, CH], f8, isOutput=False)
        wq8 = nc.declare_dram_parameter("wq8", [128, CCG, 2, H], f8, isOutput=False)
        wk8 = nc.declare_dram_parameter("wk8", [128, CCG, 2, H], f8, isOutput=False)
    else:
        wqT = nc.declare_dram_parameter("wqT", [128, CC, H], p_dt, isOutput=False)
        wkT = nc.declare_dram_parameter("wkT", [128, CC, H], p_dt, isOutput=False)
    wvT = nc.declare_dram_parameter("wvT", [128, CC, H], p_dt, isOutput=False)
    msk = nc.declare_dram_parameter("msk", [128, 128], a_dt, isOutput=False)
    eye = nc.declare_dram_parameter("eye", [128, 128], a_dt, isOutput=False)
    out = nc.declare_dram_parameter("out", [T, H], dt.float32, isOutput=True)

    with tile.TileContext(nc) as tc:
        with (
            tc.tile_pool(name="singles", bufs=1) as singles,
            tc.tile_pool(name="xp", bufs=3) as xp,
            tc.tile_pool(name="qtp", bufs=2) as qtp,
            tc.tile_pool(name="ktp", bufs=4) as ktp,
            tc.tile_pool(name="vtp", bufs=2) as vtp,
            tc.tile_pool(name="ptp", bufs=18) as ptp,
            tc.tile_pool(name="outp", bufs=4) as outp,
            tc.tile_pool(name="recp", bufs=4) as recp,
            tc.tile_pool(name="psq", bufs=2, space="PSUM") as psq,
            tc.tile_pool(name="pss", bufs=2, space="PSUM") as pss,
            tc.tile_pool(name="psv", bufs=1, space="PSUM") as psv,
            tc.tile_pool(name="pst", bufs=1, space="PSUM") as pst,
        ):
            # PE warmup on zeroed tiles (HAM clock release). memsets on
            # GpSimd so the chain starts right after the framework barrier,
            # while the first x/weight DMAs are still in flight.
            wu_a = singles.tile([128, 128], a_dt)
            wu_b = singles.tile([128, CH], a_dt)
            wu_a_ap, wu_b_ap = wu_a[:], wu_b[:]
            if ATT_DT == "float32r":
                wu_a_ap = wu_a_ap.bitcast(dt.float32)
                wu_b_ap = wu_b_ap.bitcast(dt.float32)
            nc.gpsimd.memset(wu_a_ap, 0.0)
            nc.gpsimd.memset(wu_b_ap, 0.0)
            wu_ps = psq.tile([128, CH], dt.float32, tag="pq")
            for i in range(NWU):
                nc.tensor.matmul(
                    wu_ps[:], wu_a[:], wu_b[:],
                    start=(i == 0), stop=(i == NWU - 1),
                )

            # --- weights / constants: one queue per tensor so the triggers
            # don't serialize and wk/wv aren't stuck behind the x stream ---
            if QK_FP8:
                wq_sb = singles.tile([128, CCG, 2, H], f8)
                wk_sb = singles.tile([128, CCG, 2, H], f8)
                for g0 in range(0, CCG, 2):
                    nc.gpsimd.dma_start(
                        out=wq_sb[:, g0 : g0 + 2], in_=wq8[:, g0 : g0 + 2]
                    )
                nc.scalar.dma_start(out=wk_sb[:], in_=wk8[:])
            else:
                wq_sb = singles.tile([128, CC, H], p_dt)
                wk_sb = singles.tile([128, CC, H], p_dt)
                for g0 in range(0, CC, 2):
                    nc.gpsimd.dma_start(
                        out=wq_sb[:, g0 : g0 + 2, :], in_=wqT[:, g0 : g0 + 2, :]
                    )
            wv_sb = singles.tile([128, CC, H], p_dt)
            eye_sb = singles.tile([128, 128], a_dt)
            msk_sb = singles.tile([128, 128], a_dt)
            nc.gpsimd.dma_start(out=eye_sb[:], in_=eye[:])
            nc.gpsimd.dma_start(out=msk_sb[:], in_=msk[:])

            # V' = [V | 1]; ones columns written once
            v_sb = singles.tile([128, NT, H + 4], a_dt)
            ones_ap = v_sb[:, :, H : H + 2]
            if ATT_DT == "float32r":
                ones_ap = ones_ap.bitcast(dt.float32)
            nc.vector.memset(ones_ap, 1.0)

            # persistent slotted PSUM banks
            pv_acc = psv.tile([128, 512], dt.float32)   # 3 slots x 130
            tr_acc = pst.tile([128, 512], a_dt)         # 4 slots x 128

            kt_tiles = []
            pts_all = []   # per chunk: list of pt pair tiles [128, 2*CH]

            def emit_chains(qc):
                """PV chains + normalize + store for q-chunk qc. The last
                chunk's chains mostly use the (by then idle) psq banks: a
                chain's start=True clobbers its bank's zero-region, so the
                framework serializes it behind all readers of that bank —
                alternating banks keeps the tail chains off that WAR."""
                pts_c = pts_all[qc]
                for ti in range(4):
                    qi = qc * 4 + ti
                    if qc == NCH - 1 and ti != 2:
                        opst = psq.tile([128, CH], dt.float32, tag="pq",
                                        name="opst")
                        ops = opst[:, 0:130]
                    else:
                        slot = qi % 3
                        ops = pv_acc[:, slot * 130 : slot * 130 + 130]
                    for j2 in range(qi + 1):
                        pt_pair = pts_c[j2 // 2]
                        c0 = (j2 % 2) * CH + ti * 128
                        nc.tensor.matmul(
                            ops[:, 0 : H + 2],
                            pt_pair[:, c0 : c0 + 128],
                            v_sb[:, j2, 0 : H + 2],
                            start=(j2 == 0), stop=(j2 == qi),
                        )
                    rec = recp.tile([128, 1], dt.float32)
                    nc.vector.reciprocal(rec[:], ops[:, H : H + 1])
                    ob = outp.tile([128, H], dt.float32)
                    nc.vector.tensor_scalar_mul(ob[:], ops[:, 0:H], rec[:])
                    eng = nc.sync if (qi % 2 == 0) else nc.gpsimd
                    eng.dma_start(
                        out=out[qi * 128 : (qi + 1) * 128, :], in_=ob[:]
                    )

            xts = {}
            qts = {}

            def emit_xload(qc):
                """x chunk DMA; chunk 0 in small pieces so the first
                projection matmuls start as soon as possible."""
                if QK_FP8:
                    xt8 = xp.tile([128, CCG, 2, CH], f8, tag="x8")
                    b8 = [0, 1, 2, 3, 4] if qc == 0 else [0, 2, 4]
                    for a, b in zip(b8, b8[1:]):
                        nc.sync.dma_start(out=xt8[:, a:b], in_=xq8[qc, :, a:b])
                    xts[("x8", qc)] = xt8
                xt = xp.tile([128, CC, CH], p_dt)
                bounds = ([0, 2, 4, 6, 8] if QK_FP8 else [0, 1, 2, 3, 4, 6, 8]) \
                    if qc == 0 else [0, 4, 8]
                for a, b in zip(bounds, bounds[1:]):
                    nc.sync.dma_start(out=xt[:, a:b, :], in_=xh[qc, :, a:b, :])
                xts[qc] = xt

            def _proj(w_sb, xt, xt8):
                ps = psq.tile([128, CH], dt.float32, tag="pq", name="ps")
                if QK_FP8:
                    for g in range(CCG):
                        nc.tensor.matmul(
                            ps[:], w_sb[:, g], xt8[:, g],
                            start=(g == 0), stop=(g == CCG - 1),
                            perf_mode=mybir.MatmulPerfMode.DoubleRow,
                        )
                else:
                    for cc in range(CC):
                        nc.tensor.matmul(
                            ps[:], w_sb[:, cc, :], xt[:, cc, :],
                            start=(cc == 0), stop=(cc == CC - 1),
                        )
                return ps

            def emit_qkproj(qc):
                xt, xt8 = xts[qc], xts.get(("x8", qc))
                qps = _proj(wq_sb, xt, xt8)
                qt = qtp.tile([128, CH], a_dt)
                nc.vector.tensor_copy(qt[:], qps[:])
                qts[qc] = qt
                kps = _proj(wk_sb, xt, xt8)
                kt = ktp.tile([128, CH], a_dt)
                nc.vector.tensor_copy(kt[:], kps[:])
                kt_tiles.append(kt)

            def emit_spairs(qc, p0, p1):
                """S^T pairs [p0, p1): two full-width j-tiles per [128,1024]
                PSUM tile, one exp per pair. Diagonal blocks get -1024 added
                to their strictly-masked entries via an extra matmul."""
                qt = qts[qc]
                pts_c = pts_all[qc]
                for p in range(p0, p1):
                    sp = pss.tile([128, 2 * CH], dt.float32)
                    pt = ptp.tile([128, 2 * CH], a_dt)
                    diag_pair = 2 * p >= qc * 4
                    for hh in range(2):
                        jt = 2 * p + hh
                        kt_src = kt_tiles[jt // 4]
                        v0 = (jt - qc * 4) * 128 if diag_pair else 0
                        nc.tensor.matmul(
                            sp[:, hh * CH + v0 : (hh + 1) * CH],
                            kt_src[:, (jt % 4) * 128 : (jt % 4 + 1) * 128],
                            qt[:, v0:CH],
                            start=True, stop=not diag_pair,
                        )
                        if diag_pair:
                            b0 = hh * CH + v0
                            nc.tensor.matmul(
                                sp[:, b0 : b0 + 128],
                                eye_sb[:], msk_sb[:],
                                start=False, stop=True,
                            )
                            nc.scalar.activation(
                                pt[:, b0 : (hh + 1) * CH],
                                sp[:, b0 : (hh + 1) * CH],
                                mybir.ActivationFunctionType.Exp,
                                scale=float(EXP_SCALE),
                            )
                    if not diag_pair:
                        nc.scalar.activation(
                            pt[:], sp[:], mybir.ActivationFunctionType.Exp,
                            scale=float(EXP_SCALE),
                        )
                    pts_c.append(pt)

            def emit_vproj(qc):
                xt = xts[qc]
                vps = psq.tile([128, CH], dt.float32, tag="pq", name="vps")
                for cc in range(CC):
                    nc.tensor.matmul(
                        vps[:], wv_sb[:, cc, :], xt[:, cc, :],
                        start=(cc == 0), stop=(cc == CC - 1),
                    )
                vt = vtp.tile([128, CH], a_dt)
                nc.vector.tensor_copy(vt[:], vps[:])
                for ti in range(4):
                    jt = qc * 4 + ti
                    dst = tr_acc[:, ti * 128 : (ti + 1) * 128]
                    nc.tensor.transpose(
                        dst, vt[:, ti * 128 : (ti + 1) * 128], eye_sb[:]
                    )
                    nc.vector.tensor_copy(v_sb[:, jt, 0:H], dst)

            pts_all.extend([[] for _ in range(NCH)])
            # Explicit schedule: chunk qc's PV chains run one chunk later so
            # the PE never waits on exp latency; chunk 3's off-diagonal S
            # pairs are pulled into chunk 2 so only its two diagonal pairs
            # gate the final chains.
            emit_xload(0)
            # wk/wv on the same sync ring, ORDERED AFTER chunk-0's x pieces:
            # x0 gets the full early-ramp bandwidth; wk still lands before
            # k-proj needs it. (fp8 mode loads wk in the weights block.)
            if not QK_FP8:
                nc.sync.dma_start(out=wk_sb[:], in_=wkT[:])
            nc.sync.dma_start(out=wv_sb[:], in_=wvT[:])
            for qc in range(1, 3):
                emit_xload(qc)
            emit_qkproj(0); emit_spairs(0, 0, 2); emit_vproj(0)
            emit_qkproj(1); emit_spairs(1, 0, 4); emit_chains(0); emit_vproj(1)
            emit_qkproj(2); emit_spairs(2, 0, 6)
            emit_xload(3)
            emit_qkproj(3); emit_spairs(3, 0, 6)
            emit_chains(1); emit_vproj(2)
            emit_chains(2); emit_spairs(3, 6, 8); emit_vproj(3)
            emit_chains(3)

    nc.compile()
    return nc


def _get_nc():
    if "nc" not in _CACHE:
        _CACHE["nc"] = _build()
    return _CACHE["nc"]


def _np_dt(name):
    if name == "bfloat16":
        import ml_dtypes

        return ml_dtypes.bfloat16
    return np.float32


def _in_maps(x, Wq, Wk, Wv):
    pdt = _np_dt(PROJ_DT)
    adt = _np_dt(ATT_DT)

    def _wprep(W):
        # W [H, D] -> [128p, CC, H] with per-partition-contiguous rows
        WT = np.asarray(W, dtype=np.float32).T.reshape(CC, 128, H)
        return np.ascontiguousarray(WT.transpose(1, 0, 2)).astype(pdt)

    wv = _wprep(Wv)
    # msk[j, q] = MSK_HOST where q < j (strictly masked in the diagonal block)
    msk = (MSK_HOST * np.tril(np.ones((128, 128), dtype=np.float32), -1)).astype(adt)
    eye = np.eye(128, dtype=np.float32).astype(adt)
    x = np.asarray(x, dtype=np.float32)

    if QK_FP8:
        import ml_dtypes

        f8 = getattr(ml_dtypes, "float8_e4m3", None) or ml_dtypes.float8_e4m3fn

        def _wprep8(W):
            WT = (np.float32(QKS) * np.asarray(W, np.float32)).T  # [D, H]
            WT = WT.reshape(CCG, 2, 128, H).transpose(2, 0, 1, 3)
            return np.ascontiguousarray(WT).astype(f8)

        wq8, wk8 = _wprep8(Wq), _wprep8(Wk)
    else:
        wq, wk = _wprep(Wq), _wprep(Wk)

    maps = []
    for b in range(B):
        # [qc, p, cc, t]: per (qc, p) a contiguous CC*CH run
        xh = np.ascontiguousarray(
            x[b].T.reshape(CC, 128, NCH, CH).transpose(2, 1, 0, 3)
        ).astype(pdt)
        m = {"xh": xh, "wvT": wv, "msk": msk, "eye": eye}
        if QK_FP8:
            m["xq8"] = np.ascontiguousarray(
                x[b].T.reshape(CCG, 2, 128, NCH, CH).transpose(3, 2, 0, 1, 4)
            ).astype(f8)
            m["wq8"], m["wk8"] = wq8, wk8
        else:
            m["wqT"], m["wkT"] = wq, wk
        maps.append(m)
    return maps


def kernel(x, Wq, Wk, Wv):
    from concourse.bass_utils import run_bass_kernel_spmd

    nc = _get_nc()
    res = run_bass_kernel_spmd(nc, _in_maps(x, Wq, Wk, Wv), core_ids=list(range(B)))
    return np.stack([res.results[b]["out"] for b in range(B)]).astype(np.float32)
